# revision 1
# baseline (speedup 1.0000x reference)
"""Bass/Tile kernel for nn_GatedAttention: Q-sharded attention + replicated GRU.

Per-core layouts are documented/validated in model.py. Folded scalings (exact):
  - pwT_d pre-scaled by 0.5 on host; xT_p = (tanh_g + 1) * pwT      [= gt*pw]
  - ct_half = 0.5 * ct via ACT copy; xT_c = (tanh_g + 1) * ct_half  [= gt*ct]
  - WhhT hn-part and bhn pre-scaled 0.5; rn = (tanh_r + 1) * psC    [= r*hn]
  - h' = 0.5*[(tanh_z + 1) * h  -  (tanh_z - 1) * n]
"""
import numpy as np

import concourse.bacc as bacc
import concourse.bass as bass
import concourse.mybir as mybir
import concourse.tile as tile
from concourse.masks import make_identity

F32 = mybir.dt.float32
AF = mybir.ActivationFunctionType
ALU = mybir.AluOpType

N_CORES = 8
D = 512
H4 = 1024
H3 = 1536

CONST_SHAPES = {
    "aQT": (128, 2048), "qnT": (128, 2048),
    "WvT": (128, 2048), "WvpT": (128, 2048), "WgcT": (128, 4096),
    "WihT": (128, 8 * H3), "WhhT": (128, 4 * H3),
    "brz": (1, H4), "binn": (1, D), "bhn": (1, D),
}


def host_prep_core(passage, question, Wuq, Wup, Wvp, Wv, Wg, W_ih, W_hh,
                   b_ih, b_hh, q0):
    """Per-core numpy inputs (all partition-major)."""
    B, P, _ = passage.shape
    a = (question @ Wuq.T).astype(np.float32)        # (B,Q,D)
    b_all = (passage.reshape(-1, D) @ Wup.T).reshape(B, P, D).astype(np.float32)
    G_p = (passage.reshape(-1, D) @ Wg[:, :D].T).reshape(B, P, H4).astype(np.float32)

    def attT(x):  # (B, 8, D) -> (128, 2048): [p, 512i + ql*64 + b]
        y = x.transpose(2, 1, 0).reshape(4, 128, 8, 64)
        return np.ascontiguousarray(y.transpose(1, 0, 2, 3).reshape(128, 2048))

    def stepT(x):  # (B, P, F) -> (P, 128, F//2): [t, p, 64i+b]
        F = x.shape[2]
        y = x.transpose(1, 2, 0).reshape(P, F // 128, 128, 64)
        return np.ascontiguousarray(y.transpose(0, 2, 1, 3).reshape(P, 128, F // 2))

    def wT(W):  # (E, K) -> (128, (K//128)*E): [p, E*i + e] = W[e, 128i+p]
        E, K = W.shape
        y = W.T.reshape(K // 128, 128, E)
        return np.ascontiguousarray(y.transpose(1, 0, 2).reshape(128, (K // 128) * E))

    W_hh_s = W_hh.copy()
    W_hh_s[2 * D:, :] *= 0.5
    return dict(
        aQT=attT(a[:, q0:q0 + 8, :]),
        qnT=attT(question[:, q0:q0 + 8, :]),
        bbT=stepT(b_all),
        gpT=stepT(G_p),
        pwT=stepT(passage.astype(np.float32) * 0.5),
        WvT=wT(Wv), WvpT=wT(Wvp), WgcT=wT(Wg[:, D:]),
        WihT=wT(W_ih), WhhT=wT(W_hh_s),
        brz=(b_ih + b_hh)[:H4].reshape(1, H4).astype(np.float32),
        binn=b_ih[H4:].reshape(1, D).astype(np.float32),
        bhn=(0.5 * b_hh[H4:]).reshape(1, D).astype(np.float32),
    )


def build(P):
    nc = bacc.Bacc("TRN2", target_bir_lowering=False, debug=False,
                   num_devices=N_CORES)
    dram = {k: nc.dram_tensor(k, list(s), F32, kind="ExternalInput")
            for k, s in CONST_SHAPES.items()}
    for k, s in (("bbT", (P, 128, 256)), ("gpT", (P, 128, 512)),
                 ("pwT", (P, 128, 256))):
        dram[k] = nc.dram_tensor(k, list(s), F32, kind="ExternalInput")
    out = nc.dram_tensor("out", [64, P, D], F32, kind="ExternalOutput")

    with tile.TileContext(nc) as tc:
        with (
            tc.tile_pool(name="const", bufs=1) as cpool,
            tc.tile_pool(name="wp", bufs=2) as wp,
            tc.tile_pool(name="wp1", bufs=1) as wp1,
            tc.tile_pool(name="pf", bufs=3) as pf,
            tc.tile_pool(name="pf2", bufs=2) as pf2,
            tc.tile_pool(name="state", bufs=2) as sp,
            tc.tile_pool(name="ps_u", bufs=1, space="PSUM") as ps_u,
            tc.tile_pool(name="ps_s", bufs=2, space="PSUM") as ps_s,
            tc.tile_pool(name="ps_lt", bufs=1, space="PSUM") as ps_lt,
            tc.tile_pool(name="ps_a", bufs=1, space="PSUM") as ps_a,
            tc.tile_pool(name="ps_b", bufs=1, space="PSUM") as ps_b,
            tc.tile_pool(name="ps_c", bufs=1, space="PSUM") as ps_c,
            tc.tile_pool(name="dram", bufs=2, space="DRAM") as dp,
        ):
            # ---- constants resident in SBUF
            cs = {}
            for k in CONST_SHAPES:
                t_ = cpool.tile(list(CONST_SHAPES[k]), F32, tag=k)
                nc.sync.dma_start(t_[:], dram[k][:])
                cs[k] = t_
            ones1 = cpool.tile([1, 64], F32, tag="ones1")
            nc.vector.memset(ones1[:], 1.0)
            i64 = cpool.tile([64, 64], F32, tag="i64")
            make_identity(nc, i64[:])
            i128 = cpool.tile([128, 128], F32, tag="i128")
            make_identity(nc, i128[:])

            def wv_tile(name, i, m, E):   # ws lhsT tile (K-slice i, M-slice m)
                return cs[name][:, E * i + 128 * m: E * i + 128 * m + 128]

            # ---- state
            h_sb = sp.tile([64, D], F32, tag="h")
            hT_sb = sp.tile([128, 256], F32, tag="hT")
            nc.vector.memset(h_sb[:], 0.0)
            nc.vector.memset(hT_sb[:], 0.0)

            for t in range(P):
                # ---- per-step input prefetch
                bb = pf.tile([128, 256], F32, tag="bb")
                nc.sync.dma_start(bb[:], dram["bbT"][t])
                gp = pf2.tile([128, 512], F32, tag="gp")
                nc.sync.dma_start(gp[:], dram["gpT"][t])
                pw = pf.tile([128, 256], F32, tag="pw")
                nc.sync.dma_start(pw[:], dram["pwT"][t])

                # ---- A: psu = bb + Wvp-ws @ hT     (u.T packed (128, 4*64))
                psu = ps_u.tile([128, 256], F32, tag="psu")
                nc.tensor.matmul(psu[:], i128[:], bb[:], start=True, stop=False,
                                 skip_group_check=True)
                for m in range(4):
                    for i in range(4):
                        nc.tensor.matmul(
                            psu[:, 64 * m:64 * m + 64],
                            wv_tile("WvpT", i, m, 512),
                            hT_sb[:, 64 * i:64 * i + 64],
                            start=False, stop=(m == 3 and i == 3),
                            skip_group_check=True)

                # ---- B: argT / tanh per d-tile
                tanhT = wp1.tile([128, 2048], F32, tag="tanhT")
                for i in range(4):
                    arg = wp.tile([128, 512], F32, tag="arg")
                    nc.vector.tensor_add(
                        arg[:].rearrange("p (q b) -> p q b", q=8),
                        cs["aQT"][:, 512 * i:512 * (i + 1)].rearrange(
                            "p (q b) -> p q b", q=8),
                        psu[:, 64 * i:64 * i + 64].unsqueeze(1)
                            .broadcast_to((128, 8, 64)))
                    nc.scalar.activation(tanhT[:, 512 * i:512 * (i + 1)], arg[:],
                                         AF.Tanh)

                # ---- C..I: attention per e-tile m
                ctp = wp.tile([128, 256], F32, tag="ctp")
                for m in range(4):
                    ps = ps_s.tile([128, 512], F32, tag="s")
                    for i in range(4):
                        nc.tensor.matmul(ps[:], wv_tile("WvT", i, m, 512),
                                         tanhT[:, 512 * i:512 * (i + 1)],
                                         start=(i == 0), stop=(i == 3))
                    E_m = wp.tile([128, 512], F32, tag="E")
                    nc.scalar.activation(E_m[:], ps[:], AF.Exp)
                    Z_m = wp.tile([128, 8], F32, tag="Z")
                    nc.vector.reduce_sum(
                        Z_m[:], E_m[:].rearrange("p (q b) -> p q b", q=8),
                        axis=mybir.AxisListType.X)
                    R_m = wp.tile([128, 8], F32, tag="R")
                    nc.vector.reciprocal(R_m[:], Z_m[:])
                    W2 = wp.tile([128, 512], F32, tag="W2")
                    nc.vector.tensor_mul(W2[:], E_m[:],
                                         cs["qnT"][:, 512 * m:512 * (m + 1)])
                    W3 = wp.tile([128, 512], F32, tag="W3")
                    nc.vector.tensor_mul(
                        W3[:].rearrange("p (q b) -> p q b", q=8),
                        W2[:].rearrange("p (q b) -> p q b", q=8),
                        R_m[:].unsqueeze(2).broadcast_to((128, 8, 64)))
                    nc.vector.reduce_sum(
                        ctp[:, 64 * m:64 * m + 64],
                        W3[:].rearrange("p (q b) -> p b q", q=8),
                        axis=mybir.AxisListType.X)

                # ---- J: AllReduce ct partials
                bin_ = dp.tile([128, 256], F32, tag="bin")
                bout = dp.tile([128, 256], F32, tag="bout")
                nc.sync.dma_start(bin_[:], ctp[:])
                nc.gpsimd.collective_compute(
                    "AllReduce", ALU.add,
                    replica_groups=[list(range(N_CORES))],
                    ins=[bin_.opt()], outs=[bout.opt()])
                ct = wp1.tile([128, 256], F32, tag="ct")
                nc.sync.dma_start(ct[:], bout[:])
                ct_half = wp1.tile([128, 256], F32, tag="ct_half")
                nc.scalar.activation(ct_half[:], ct[:], AF.Copy, scale=0.5)

                # ---- K: gates  lt = gp + Wgc-ws @ ct   (packed (128, 8*64))
                pslt = ps_lt.tile([128, 512], F32, tag="pslt")
                nc.tensor.matmul(pslt[:], i128[:], gp[:], start=True, stop=False,
                                 skip_group_check=True)
                for m in range(8):
                    for i in range(4):
                        nc.tensor.matmul(
                            pslt[:, 64 * m:64 * m + 64],
                            wv_tile("WgcT", i, m, 1024),
                            ct[:, 64 * i:64 * i + 64],
                            start=False, stop=(m == 7 and i == 3),
                            skip_group_check=True)
                # ---- L: tanh_g = tanh(lt/2);  xT = (tanh_g+1)*[pw | ct_half]
                tg = wp1.tile([128, 512], F32, tag="tg")
                nc.scalar.activation(tg[:], pslt[:], AF.Tanh, scale=0.5)
                xT = wp1.tile([128, 512], F32, tag="xT")
                nc.vector.scalar_tensor_tensor(
                    xT[:, 0:256], tg[:, 0:256], 1.0, pw[:],
                    op0=ALU.add, op1=ALU.mult)
                nc.vector.scalar_tensor_tensor(
                    xT[:, 256:512], tg[:, 256:512], 1.0, ct_half[:],
                    op0=ALU.add, op1=ALU.mult)

                # ---- N: GRU matmuls
                psA = ps_a.tile([64, H4], F32, tag="psA")   # r|z logits
                psB = ps_b.tile([64, D], F32, tag="psB")    # inn
                psC = ps_c.tile([64, D], F32, tag="psC")    # 0.5*hn
                for c in range(2):
                    nc.tensor.matmul(psA[:, 512 * c:512 * (c + 1)], ones1[:],
                                     cs["brz"][:, 512 * c:512 * (c + 1)],
                                     start=True, stop=False, skip_group_check=True)
                nc.tensor.matmul(psB[:], ones1[:], cs["binn"][:],
                                 start=True, stop=False, skip_group_check=True)
                nc.tensor.matmul(psC[:], ones1[:], cs["bhn"][:],
                                 start=True, stop=False, skip_group_check=True)
                for i in range(8):   # gi
                    lhsT = xT[:, 64 * i:64 * i + 64]
                    base = H3 * i
                    for c in range(2):
                        nc.tensor.matmul(
                            psA[:, 512 * c:512 * (c + 1)], lhsT,
                            cs["WihT"][:, base + 512 * c: base + 512 * (c + 1)],
                            start=False, stop=False, skip_group_check=True)
                    nc.tensor.matmul(
                        psB[:], lhsT, cs["WihT"][:, base + 1024: base + 1536],
                        start=False, stop=(i == 7), skip_group_check=True)
                for i in range(4):   # gh
                    lhsT = hT_sb[:, 64 * i:64 * i + 64]
                    base = H3 * i
                    last = (i == 3)
                    for c in range(2):
                        nc.tensor.matmul(
                            psA[:, 512 * c:512 * (c + 1)], lhsT,
                            cs["WhhT"][:, base + 512 * c: base + 512 * (c + 1)],
                            start=False, stop=(last and c == 1),
                            skip_group_check=True)
                    nc.tensor.matmul(
                        psC[:], lhsT, cs["WhhT"][:, base + 1024: base + 1536],
                        start=False, stop=last, skip_group_check=True)

                # ---- O: gate nonlinearities + state update
                trz = wp1.tile([64, H4], F32, tag="trz")
                nc.scalar.activation(trz[:], psA[:], AF.Tanh, scale=0.5)
                rn = wp1.tile([64, D], F32, tag="rn")
                nc.vector.scalar_tensor_tensor(
                    rn[:], trz[:, 0:512], 1.0, psC[:],
                    op0=ALU.add, op1=ALU.mult)
                npre = wp1.tile([64, D], F32, tag="npre")
                nc.vector.tensor_add(npre[:], rn[:], psB[:])
                n_sb = wp1.tile([64, D], F32, tag="n")
                nc.scalar.activation(n_sb[:], npre[:], AF.Tanh)
                ta = wp1.tile([64, D], F32, tag="ta")
                nc.vector.scalar_tensor_tensor(
                    ta[:], trz[:, 512:1024], 1.0, h_sb[:],
                    op0=ALU.add, op1=ALU.mult)
                tb = wp1.tile([64, D], F32, tag="tb")
                nc.vector.scalar_tensor_tensor(
                    tb[:], trz[:, 512:1024], -1.0, n_sb[:],
                    op0=ALU.add, op1=ALU.mult)
                hdiff = wp1.tile([64, D], F32, tag="hdiff")
                nc.vector.tensor_sub(hdiff[:], ta[:], tb[:])
                h_new = sp.tile([64, D], F32, tag="h")
                nc.vector.tensor_scalar_mul(h_new[:], hdiff[:], 0.5)

                # ---- P: output
                nc.sync.dma_start(out[:, t, :], h_new[:])

                # ---- Q: hT for next step
                pstr = ps_u.tile([128, 256], F32, tag="psu")
                for i in range(4):
                    nc.tensor.transpose(pstr[:, 64 * i:64 * i + 64],
                                        h_new[:, 128 * i:128 * (i + 1)], i64[:])
                hT_new = sp.tile([128, 256], F32, tag="hT")
                nc.scalar.activation(hT_new[:], pstr[:], AF.Copy)

                h_sb = h_new
                hT_sb = hT_new

    nc.compile()
    return nc


def make_in_maps(inputs, P):
    """inputs: dict of full numpy arrays as from setup_inputs(). Returns in_maps."""
    passage = np.asarray(inputs["passage"], dtype=np.float32)[:, :P]
    question = np.asarray(inputs["question"], dtype=np.float32)
    ws = {k: np.asarray(inputs[k], dtype=np.float32) for k in
          ("Wuq", "Wup", "Wvp", "Wv", "Wg", "W_ih", "W_hh", "b_ih", "b_hh")}
    in_maps = []
    for core in range(N_CORES):
        pr = host_prep_core(passage, question, ws["Wuq"], ws["Wup"], ws["Wvp"],
                            ws["Wv"], ws["Wg"], ws["W_ih"], ws["W_hh"],
                            ws["b_ih"], ws["b_hh"], q0=8 * core)
        in_maps.append(pr)
    return in_maps


_CACHE = {}


def kernel(**inputs):
    """Full-input entrypoint: shards internally across 8 NeuronCores."""
    from concourse.bass_utils import run_bass_kernel_spmd
    passage = np.asarray(inputs["passage"], dtype=np.float32)
    P = passage.shape[1]
    if P not in _CACHE:
        _CACHE[P] = build(P)
    nc = _CACHE[P]
    in_maps = make_in_maps(inputs, P)
    r = run_bass_kernel_spmd(nc, in_maps, core_ids=list(range(N_CORES)))
    return np.asarray(r.results[0]["out"], dtype=np.float32)

